# revision 9
# baseline (speedup 1.0000x reference)
"""Cross-attention (Bahdanau-style) scores kernel for 8 Trainium2 NeuronCores.

Reference computation (per batch b, source position s):
    energy[b,s,:] = tanh(Wh @ h[b] + We @ eo[s,b] + bias)
    scores[b,s]   = v . energy[b,s,:]
    out[b,:]      = softmax(scores[b,:])   over s

Sharding: data-parallel over batch (64 batches -> 8 per core). Weights are
replicated. No collectives needed (softmax is per-batch, fully local).

Per-core pipeline (S=4096, Bc=8, E2=512, D=256), v4:
  - host pre-transposes eo to [bb, g, h, p, c, j] bf16 so every DMA is a
    fully contiguous 512 KiB block (4 KiB per partition -> near-peak HBM BW)
  - DMA issue spread across Vector/Scalar/Sync queues so the first tiles
    land ~6.5us instead of ~15us (Sync alone has a 7us runtime preamble)
  - dummy matmuls on a memset tile warm the PE clock (HAM) from ~5us
  - PE matmul (bf16, N=512): PSUM [128, 2, 512], pep bufs=3
  - ACT: energy = tanh(eprojT + baseT[k]) fused, one instr per 1024 cols
  - PE dot (lag-3 slots, pair-batched, flushed per group):
    scores[8, s512] += vm[k,bb].T @ energy[k, s]
  - eager DVE copies PSUM->SBUF + eager ACT exp per 512-range
  - tail: last exp, 1/sum, scale split across ACT+DVE, two output DMAs.
"""

import numpy as np
import ml_dtypes

import concourse.bass as bass
import concourse.bacc as bacc
import concourse.tile as tile
from concourse import mybir
from concourse.bass_utils import run_bass_kernel_spmd

dt = mybir.dt

S = 4096          # src_len
B = 64            # global batch
E2 = 512          # 2*enc_hid
D = 256           # dec_hid
NCORES = 8
BC = B // NCORES  # batches per core = 8
P = 128
SG = 1024         # per-(bb,g) s-extent
NG = S // SG      # 4 s-groups
HW = 512          # matmul free-dim width (PSUM bank limit)
NH = SG // HW     # 2 halves per s-group
NR = S // HW      # 8 score ranges
NEC = E2 // P     # 4 e-chunks
NKC = D // P      # 2 k-chunks
LAG = 3           # dot emission lag in (bb,kc) slots
NWARM = 26        # PE warm-up dummy matmuls (bridge until data lands)

F32 = dt.float32
BF16 = dt.bfloat16


def build_program():
    nc = bacc.Bacc(None, target_bir_lowering=False, debug=False, num_devices=8)

    # eoT[bb, g, h, p, c, j] = eo[g*1024+h*512+j, bb, c*128+p]  (bf16)
    eoT_d = nc.declare_dram_parameter(
        "eoT", [BC, NG, NH, P, NEC, HW], BF16, isOutput=False)
    # WeT_r[p, ec, k] = We.T[ec*128+p, k] ; We = W[:, D:]
    weT_d = nc.declare_dram_parameter("weT", [P, NEC, D], BF16, isOutput=False)
    # WhT_r[p, dc, kc, j] = W[kc*128+j, dc*128+p]  (Wh part, pre-chunked)
    whT_d = nc.declare_dram_parameter("whT", [P, NKC, NKC, P], F32, isOutput=False)
    # hT[p, dc, bb] = h[bb, dc*128+p]
    hT_d = nc.declare_dram_parameter("hT", [P, NKC, BC], F32, isOutput=False)
    # bT[p, kc] = bias[kc*128+p]
    bT_d = nc.declare_dram_parameter("bT", [P, NKC], F32, isOutput=False)
    # vm[p, kc, bb, m] = v[kc*128+p] if m == bb else 0  (dot -> partition bb)
    vm_d = nc.declare_dram_parameter("vm", [P, NKC, BC, BC], BF16, isOutput=False)
    out_d = nc.declare_dram_parameter("out", [BC, S], F32, isOutput=True)

    with tile.TileContext(nc) as tc:
        with (
            tc.tile_pool(name="consts", bufs=1) as consts,
            tc.tile_pool(name="eot", bufs=6) as eot_pool,
            tc.tile_pool(name="en", bufs=6) as en_pool,
            tc.tile_pool(name="pep", bufs=3, space="PSUM") as pep_pool,
            tc.tile_pool(name="psc", bufs=2, space="PSUM") as psc_pool,
        ):
            # ---- PE warm-up: memset a tile, run dummy matmuls on it ----
            warm = consts.tile([P, HW], BF16)
            nc.vector.memset(warm, 0)
            ps_warm = psc_pool.tile([BC, HW], F32, tag="psc", name="ps_warm")
            for _ in range(NWARM):
                nc.tensor.matmul(ps_warm, warm[:, :BC], warm,
                                 start=True, stop=True)

            # ---- DMAs spread across queues for early arrival ----
            weT = consts.tile([P, NEC, D], BF16)
            nc.gpsimd.dma_start(out=weT, in_=weT_d[:])
            eoT_first = eot_pool.tile([P, NH, NEC, HW], BF16, tag="eot",
                                      name="eoT_first")
            for h in range(NH):
                nc.gpsimd.dma_start(out=eoT_first[:, h], in_=eoT_d[0, 0, h])

            whT = consts.tile([P, NKC, NKC, P], F32)
            nc.scalar.dma_start(out=whT, in_=whT_d[:])
            hT = consts.tile([P, NKC, BC], F32)
            nc.scalar.dma_start(out=hT, in_=hT_d[:])
            bT = consts.tile([P, NKC], F32)
            nc.scalar.dma_start(out=bT, in_=bT_d[:])

            vm = consts.tile([P, NKC, BC, BC], BF16)
            nc.sync.dma_start(out=vm, in_=vm_d[:])

            baseT = consts.tile([P, NKC, BC], F32)    # [k128, kc, bb]
            esums = consts.tile([BC, NR], F32)        # per-range exp sums
            out_sb = consts.tile([BC, S], F32)

            # --- init: baseT[k, bb] = sum_d Wh[k, d] h[bb, d] + bias[k] ---
            ps_base = pep_pool.tile([P, NKC, BC], F32, tag="pep",
                                    name="ps_base")
            for kc in range(NKC):
                for dc in range(NKC):
                    nc.tensor.matmul(
                        ps_base[:, kc, :],
                        whT[:, dc, kc, :],
                        hT[:, dc, :],
                        start=(dc == 0),
                        stop=(dc == NKC - 1),
                    )
            for kc in range(NKC):
                nc.vector.tensor_scalar_add(
                    baseT[:, kc, :], ps_base[:, kc, :], bT[:, kc : kc + 1]
                )

            pending = []  # (ps_sc, en_half_ap, kc, bb, first, last, ridx)

            def emit_dot(item):
                ps_sc, en_ap, kc, bb, first, last, ridx = item
                nc.tensor.matmul(
                    ps_sc, vm[:, kc, bb, :], en_ap, start=first, stop=last,
                )
                if last:
                    # eager: exp straight out of PSUM (frees the bank, keeps
                    # the tail chain short)
                    nc.scalar.activation(
                        out=out_sb[:, ridx * HW : (ridx + 1) * HW],
                        in_=ps_sc,
                        func=mybir.ActivationFunctionType.Exp,
                        accum_out=esums[:, ridx : ridx + 1],
                    )

            for g in range(NG):
                ps_scs = [psc_pool.tile([BC, HW], F32, tag="psc",
                                        name=f"psc_{g}_{h}")
                          for h in range(NH)]
                for bb in range(BC):
                    if g == 0 and bb == 0:
                        eoT_t = eoT_first
                    else:
                        eoT_t = eot_pool.tile([P, NH, NEC, HW], BF16,
                                              tag="eot", name=f"eoT_{g}_{bb}")
                        if g == 0 and bb in (1, 3):
                            eng = nc.scalar
                        elif bb % 2 == 0:
                            eng = nc.sync
                        else:
                            eng = nc.gpsimd
                        for h in range(NH):
                            eng.dma_start(out=eoT_t[:, h], in_=eoT_d[bb, g, h])

                    # ---- projection + fused tanh; dots emitted lagged ----
                    for kc in range(NKC):
                        ps_ep = pep_pool.tile([P, NH, HW], F32, tag="pep",
                                              name=f"pep_{g}_{bb}_{kc}")
                        for c in range(NEC):
                            for h in range(NH):
                                nc.tensor.matmul(
                                    ps_ep[:, h, :],
                                    weT[:, c, kc * P : (kc + 1) * P],
                                    eoT_t[:, h, c, :],
                                    start=(c == 0),
                                    stop=(c == NEC - 1),
                                )
                        en = en_pool.tile([P, SG], BF16, tag="en",
                                          name=f"en_{g}_{bb}_{kc}")
                        nc.scalar.activation(
                            out=en, in_=ps_ep,
                            func=mybir.ActivationFunctionType.Tanh,
                            bias=baseT[:, kc, bb : bb + 1],
                        )
                        first = bb == 0 and kc == 0
                        last = bb == BC - 1 and kc == NKC - 1
                        for h in range(NH):
                            pending.append(
                                (ps_scs[h], en[:, h * HW : (h + 1) * HW],
                                 kc, bb, first, last, g * NH + h)
                            )
                        # pair-batched: drain two slots' dots at once
                        if len(pending) >= (LAG + 2) * NH:
                            for _ in range(2 * NH):
                                emit_dot(pending.pop(0))
                # group boundary: flush this group's dots (frees psc banks
                # well before the next-but-one group needs them)
                while pending:
                    emit_dot(pending.pop(0))

            # ---- softmax tail ----
            with tc.tile_pool(name="sm", bufs=1) as sm:
                esum = sm.tile([BC, 1], F32)
                nc.vector.tensor_reduce(
                    out=esum, in_=esums, axis=mybir.AxisListType.X,
                    op=mybir.AluOpType.add,
                )
                rsum = sm.tile([BC, 1], F32)
                nc.vector.reciprocal(rsum, esum)
                # scale quarters on two engines in parallel, DMA each ASAP
                Q = S // 4
                for qi, eng_scale in ((0, "act"), (2, "dve"), (1, "act"),
                                      (3, "dve")):
                    sl = slice(qi * Q, (qi + 1) * Q)
                    if eng_scale == "act":
                        nc.scalar.activation(
                            out=out_sb[:, sl], in_=out_sb[:, sl],
                            func=mybir.ActivationFunctionType.Copy,
                            scale=rsum,
                        )
                    else:
                        nc.vector.tensor_scalar_mul(
                            out_sb[:, sl], out_sb[:, sl], rsum
                        )
                    nc.sync.dma_start(out=out_d[:, sl], in_=out_sb[:, sl])

    return nc


_nc = None


def _get_nc():
    global _nc
    if _nc is None:
        _nc = build_program()
        _nc.compile()
    return _nc


def kernel(hidden, encoder_outputs, W, b, v):
    hidden = np.asarray(hidden, dtype=np.float32)
    encoder_outputs = np.ascontiguousarray(encoder_outputs, dtype=np.float32)
    W = np.asarray(W, dtype=np.float32)
    b = np.asarray(b, dtype=np.float32)
    v = np.asarray(v, dtype=np.float32)

    # host-side prep of the small replicated weights
    We = W[:, D:]                                     # [256, 512]
    weT = np.ascontiguousarray(
        We.T.reshape(NEC, P, D).transpose(1, 0, 2)    # [p, ec, k]
    ).astype(ml_dtypes.bfloat16)
    # whT[p, dc, kc, j] = W[kc*128+j, dc*128+p]
    Wh = W[:, :D]                                     # [k, d]
    whT = np.ascontiguousarray(
        Wh.reshape(NKC, P, NKC, P).transpose(3, 2, 0, 1)  # [p(d), dc, kc, j(k)]
    )
    bT = np.ascontiguousarray(b.reshape(NKC, P).T)    # [p, kc]
    vT = np.ascontiguousarray(v.reshape(NKC, P).T)
    vm = np.zeros((P, NKC, BC, BC), dtype=np.float32)
    for bb in range(BC):
        vm[:, :, bb, bb] = vT
    vm = vm.astype(ml_dtypes.bfloat16)
    h = hidden[0]                                     # [64, 256]

    nc = _get_nc()
    eo_bf16 = encoder_outputs.astype(ml_dtypes.bfloat16)
    # [S, B, E2] -> [B, E2, S]
    eoT_full = np.ascontiguousarray(eo_bf16.transpose(1, 2, 0))
    in_maps = []
    for i in range(NCORES):
        bsl = slice(i * BC, (i + 1) * BC)
        hT_i = np.ascontiguousarray(h[bsl].T.reshape(NKC, P, BC).transpose(1, 0, 2))
        # [BC, NEC, P, NG, NH, HW] -> [BC, NG, NH, P, NEC, HW]
        eoT_i = np.ascontiguousarray(
            eoT_full[bsl].reshape(BC, NEC, P, NG, NH, HW)
            .transpose(0, 3, 4, 2, 1, 5)
        )
        in_maps.append(
            {"eoT": eoT_i, "weT": weT, "whT": whT, "hT": hT_i, "bT": bT,
             "vm": vm}
        )

    try:
        res = run_bass_kernel_spmd(nc, in_maps, list(range(NCORES)))
    except Exception:
        # transient NRT/device hiccups happen; one retry
        res = run_bass_kernel_spmd(nc, in_maps, list(range(NCORES)))
    global _last_results
    _last_results = res
    out = np.concatenate([res.results[i]["out"] for i in range(NCORES)], axis=0)
    return out


_last_results = None


if __name__ == "__main__":
    rng = np.random.default_rng(0)
    inputs = {
        "hidden": rng.standard_normal((1, B, D), dtype=np.float32),
        "encoder_outputs": rng.standard_normal((S, B, E2), dtype=np.float32),
        "W": (rng.standard_normal((D, E2 + D)) * 0.02).astype(np.float32),
        "b": (rng.standard_normal((D,)) * 0.02).astype(np.float32),
        "v": rng.random((D,), dtype=np.float32),
    }
    out = kernel(**inputs)
    print("out", out.shape, out.dtype, out.sum())


# revision 10
# speedup vs baseline: 1.0363x; 1.0363x over previous
"""Cross-attention (Bahdanau-style) scores kernel for 8 Trainium2 NeuronCores.

Reference computation (per batch b, source position s):
    energy[b,s,:] = tanh(Wh @ h[b] + We @ eo[s,b] + bias)
    scores[b,s]   = v . energy[b,s,:]
    out[b,:]      = softmax(scores[b,:])   over s

Sharding: data-parallel over batch (64 batches -> 8 per core). Weights are
replicated. No collectives needed (softmax is per-batch, fully local).

Per-core pipeline (S=4096, Bc=8, E2=512, D=256), v4:
  - host pre-transposes eo to [bb, g, h, p, c, j] bf16 so every DMA is a
    fully contiguous 512 KiB block (4 KiB per partition -> near-peak HBM BW)
  - DMA issue spread across Vector/Scalar/Sync queues so the first tiles
    land ~6.5us instead of ~15us (Sync alone has a 7us runtime preamble)
  - dummy matmuls on a memset tile warm the PE clock (HAM) from ~5us
  - PE matmul (bf16, N=512): PSUM [128, 2, 512], pep bufs=3
  - ACT: energy = tanh(eprojT + baseT[k]) fused, one instr per 1024 cols
  - PE dot (lag-3 slots, pair-batched, flushed per group):
    scores[8, s512] += vm[k,bb].T @ energy[k, s]
  - eager DVE copies PSUM->SBUF + eager ACT exp per 512-range
  - tail: last exp, 1/sum, scale split across ACT+DVE, two output DMAs.
"""

import numpy as np
import ml_dtypes

import concourse.bass as bass
import concourse.bacc as bacc
import concourse.tile as tile
from concourse import mybir
from concourse.bass_utils import run_bass_kernel_spmd

dt = mybir.dt

S = 4096          # src_len
B = 64            # global batch
E2 = 512          # 2*enc_hid
D = 256           # dec_hid
NCORES = 8
BC = B // NCORES  # batches per core = 8
P = 128
SG = 1024         # per-(bb,g) s-extent
NG = S // SG      # 4 s-groups
HW = 512          # matmul free-dim width (PSUM bank limit)
NH = SG // HW     # 2 halves per s-group
NR = S // HW      # 8 score ranges
NEC = E2 // P     # 4 e-chunks
NKC = D // P      # 2 k-chunks
LAG = 3           # dot emission lag in (bb,kc) slots
NWARM = 12        # PE warm-up dummy matmuls (bridge until data lands)

F32 = dt.float32
BF16 = dt.bfloat16


def build_program():
    nc = bacc.Bacc(None, target_bir_lowering=False, debug=False, num_devices=8)

    # eoT[bb, g, h, p, c, j] = eo[g*1024+h*512+j, bb, c*128+p]  (bf16)
    eoT_d = nc.declare_dram_parameter(
        "eoT", [BC, NG, NH, P, NEC, HW], BF16, isOutput=False)
    # WeT_r[p, ec, k] = We.T[ec*128+p, k] ; We = W[:, D:]
    weT_d = nc.declare_dram_parameter("weT", [P, NEC, D], BF16, isOutput=False)
    # WhT_r[p, dc, kc, j] = W[kc*128+j, dc*128+p]  (Wh part, pre-chunked)
    whT_d = nc.declare_dram_parameter("whT", [P, NKC, NKC, P], F32, isOutput=False)
    # hT[p, dc, bb] = h[bb, dc*128+p]
    hT_d = nc.declare_dram_parameter("hT", [P, NKC, BC], F32, isOutput=False)
    # bT[p, kc] = bias[kc*128+p]
    bT_d = nc.declare_dram_parameter("bT", [P, NKC], F32, isOutput=False)
    # vm[p, kc, bb, m] = v[kc*128+p] if m == bb else 0  (dot -> partition bb)
    vm_d = nc.declare_dram_parameter("vm", [P, NKC, BC, BC], BF16, isOutput=False)
    out_d = nc.declare_dram_parameter("out", [BC, S], F32, isOutput=True)

    with tile.TileContext(nc) as tc:
        with (
            tc.tile_pool(name="consts", bufs=1) as consts,
            tc.tile_pool(name="eot", bufs=6) as eot_pool,
            tc.tile_pool(name="en", bufs=8) as en_pool,
            tc.tile_pool(name="pep", bufs=3, space="PSUM") as pep_pool,
            tc.tile_pool(name="psc", bufs=2, space="PSUM") as psc_pool,
        ):
            # ---- PE warm-up: memset a tile, run dummy matmuls on it ----
            warm = consts.tile([P, HW], BF16)
            nc.vector.memset(warm, 0)
            ps_warm = psc_pool.tile([BC, HW], F32, tag="psc", name="ps_warm")
            for _ in range(NWARM):
                nc.tensor.matmul(ps_warm, warm[:, :BC], warm,
                                 start=True, stop=True)

            # ---- DMAs spread across queues for early arrival ----
            whT = consts.tile([P, NKC, NKC, P], F32)
            nc.scalar.dma_start(out=whT, in_=whT_d[:])
            hT = consts.tile([P, NKC, BC], F32)
            nc.scalar.dma_start(out=hT, in_=hT_d[:])
            eoT_first = eot_pool.tile([P, NH, NEC, HW], BF16, tag="eot",
                                      name="eoT_first")
            nc.gpsimd.dma_start(out=eoT_first[:, 0], in_=eoT_d[0, 0, 0])
            weT = consts.tile([P, NEC, D], BF16)
            nc.gpsimd.dma_start(out=weT, in_=weT_d[:])
            vm = consts.tile([P, NKC, BC, BC], BF16)
            nc.sync.dma_start(out=vm, in_=vm_d[:])
            bT = consts.tile([P, NKC], F32)
            nc.sync.dma_start(out=bT, in_=bT_d[:])
            nc.sync.dma_start(out=eoT_first[:, 1], in_=eoT_d[0, 0, 1])

            baseT = consts.tile([P, NKC, BC], F32)    # [k128, kc, bb]
            esums = consts.tile([BC, NR], F32)        # per-range exp sums
            out_sb = consts.tile([BC, S], F32)

            # --- init: baseT[k, bb] = sum_d Wh[k, d] h[bb, d] + bias[k] ---
            ps_base = pep_pool.tile([P, NKC, BC], F32, tag="pep",
                                    name="ps_base")
            for kc in range(NKC):
                for dc in range(NKC):
                    nc.tensor.matmul(
                        ps_base[:, kc, :],
                        whT[:, dc, kc, :],
                        hT[:, dc, :],
                        start=(dc == 0),
                        stop=(dc == NKC - 1),
                    )
            for kc in range(NKC):
                nc.vector.tensor_scalar_add(
                    baseT[:, kc, :], ps_base[:, kc, :], bT[:, kc : kc + 1]
                )

            pending = []  # (ps_sc, en_half_ap, kc, bb, first, last, ridx)

            def emit_dot(item):
                ps_sc, en_ap, kc, bb, first, last, ridx = item
                nc.tensor.matmul(
                    ps_sc, vm[:, kc, bb, :], en_ap, start=first, stop=last,
                )
                if last:
                    # eager: exp straight out of PSUM (frees the bank, keeps
                    # the tail chain short)
                    nc.scalar.activation(
                        out=out_sb[:, ridx * HW : (ridx + 1) * HW],
                        in_=ps_sc,
                        func=mybir.ActivationFunctionType.Exp,
                        accum_out=esums[:, ridx : ridx + 1],
                    )

            for g in range(NG):
                ps_scs = [psc_pool.tile([BC, HW], F32, tag="psc",
                                        name=f"psc_{g}_{h}")
                          for h in range(NH)]
                for bb in range(BC):
                    if g == 0 and bb == 0:
                        eoT_t = eoT_first
                    else:
                        eoT_t = eot_pool.tile([P, NH, NEC, HW], BF16,
                                              tag="eot", name=f"eoT_{g}_{bb}")
                        if g == 0 and bb <= 4:
                            ring = [nc.scalar, nc.gpsimd, nc.sync]
                            for h in range(NH):
                                eng = ring[(NH * bb + h) % 3]
                                eng.dma_start(out=eoT_t[:, h],
                                              in_=eoT_d[bb, g, h])
                        else:
                            eng = nc.sync if (bb % 2 == 0) else nc.gpsimd
                            for h in range(NH):
                                eng.dma_start(out=eoT_t[:, h],
                                              in_=eoT_d[bb, g, h])

                    # ---- projection + fused tanh; dots emitted lagged ----
                    for kc in range(NKC):
                        ps_ep = pep_pool.tile([P, NH, HW], F32, tag="pep",
                                              name=f"pep_{g}_{bb}_{kc}")
                        for c in range(NEC):
                            for h in range(NH):
                                nc.tensor.matmul(
                                    ps_ep[:, h, :],
                                    weT[:, c, kc * P : (kc + 1) * P],
                                    eoT_t[:, h, c, :],
                                    start=(c == 0),
                                    stop=(c == NEC - 1),
                                )
                        en = en_pool.tile([P, SG], BF16, tag="en",
                                          name=f"en_{g}_{bb}_{kc}")
                        nc.scalar.activation(
                            out=en, in_=ps_ep,
                            func=mybir.ActivationFunctionType.Tanh,
                            bias=baseT[:, kc, bb : bb + 1],
                        )
                        first = bb == 0 and kc == 0
                        last = bb == BC - 1 and kc == NKC - 1
                        for h in range(NH):
                            pending.append(
                                (ps_scs[h], en[:, h * HW : (h + 1) * HW],
                                 kc, bb, first, last, g * NH + h)
                            )
                        # batch-4: drain four slots' dots at once (fewer
                        # stationary-operand switches between weT and vm)
                        if len(pending) >= (LAG + 4) * NH:
                            for _ in range(4 * NH):
                                emit_dot(pending.pop(0))
                # group boundary: flush this group's dots (frees psc banks
                # well before the next-but-one group needs them)
                while pending:
                    emit_dot(pending.pop(0))

            # ---- softmax tail ----
            with tc.tile_pool(name="sm", bufs=1) as sm:
                esum = sm.tile([BC, 1], F32)
                nc.vector.tensor_reduce(
                    out=esum, in_=esums, axis=mybir.AxisListType.X,
                    op=mybir.AluOpType.add,
                )
                rsum = sm.tile([BC, 1], F32)
                nc.vector.reciprocal(rsum, esum)
                # scale quarters on two engines in parallel, DMA each ASAP
                Q = S // 4
                for qi, eng_scale in ((0, "act"), (2, "dve"), (1, "act"),
                                      (3, "dve")):
                    sl = slice(qi * Q, (qi + 1) * Q)
                    if eng_scale == "act":
                        nc.scalar.activation(
                            out=out_sb[:, sl], in_=out_sb[:, sl],
                            func=mybir.ActivationFunctionType.Copy,
                            scale=rsum,
                        )
                    else:
                        nc.vector.tensor_scalar_mul(
                            out_sb[:, sl], out_sb[:, sl], rsum
                        )
                    dma_eng = nc.sync if qi % 2 == 0 else nc.gpsimd
                    dma_eng.dma_start(out=out_d[:, sl], in_=out_sb[:, sl])

    return nc


_nc = None


def _get_nc():
    global _nc
    if _nc is None:
        _nc = build_program()
        _nc.compile()
    return _nc


def kernel(hidden, encoder_outputs, W, b, v):
    hidden = np.asarray(hidden, dtype=np.float32)
    encoder_outputs = np.ascontiguousarray(encoder_outputs, dtype=np.float32)
    W = np.asarray(W, dtype=np.float32)
    b = np.asarray(b, dtype=np.float32)
    v = np.asarray(v, dtype=np.float32)

    # host-side prep of the small replicated weights
    We = W[:, D:]                                     # [256, 512]
    weT = np.ascontiguousarray(
        We.T.reshape(NEC, P, D).transpose(1, 0, 2)    # [p, ec, k]
    ).astype(ml_dtypes.bfloat16)
    # whT[p, dc, kc, j] = W[kc*128+j, dc*128+p]
    Wh = W[:, :D]                                     # [k, d]
    whT = np.ascontiguousarray(
        Wh.reshape(NKC, P, NKC, P).transpose(3, 2, 0, 1)  # [p(d), dc, kc, j(k)]
    )
    bT = np.ascontiguousarray(b.reshape(NKC, P).T)    # [p, kc]
    vT = np.ascontiguousarray(v.reshape(NKC, P).T)
    vm = np.zeros((P, NKC, BC, BC), dtype=np.float32)
    for bb in range(BC):
        vm[:, :, bb, bb] = vT
    vm = vm.astype(ml_dtypes.bfloat16)
    h = hidden[0]                                     # [64, 256]

    nc = _get_nc()
    eo_bf16 = encoder_outputs.astype(ml_dtypes.bfloat16)
    # [S, B, E2] -> [B, E2, S]
    eoT_full = np.ascontiguousarray(eo_bf16.transpose(1, 2, 0))
    in_maps = []
    for i in range(NCORES):
        bsl = slice(i * BC, (i + 1) * BC)
        hT_i = np.ascontiguousarray(h[bsl].T.reshape(NKC, P, BC).transpose(1, 0, 2))
        # [BC, NEC, P, NG, NH, HW] -> [BC, NG, NH, P, NEC, HW]
        eoT_i = np.ascontiguousarray(
            eoT_full[bsl].reshape(BC, NEC, P, NG, NH, HW)
            .transpose(0, 3, 4, 2, 1, 5)
        )
        in_maps.append(
            {"eoT": eoT_i, "weT": weT, "whT": whT, "hT": hT_i, "bT": bT,
             "vm": vm}
        )

    try:
        res = run_bass_kernel_spmd(nc, in_maps, list(range(NCORES)))
    except Exception:
        # transient NRT/device hiccups happen; one retry
        res = run_bass_kernel_spmd(nc, in_maps, list(range(NCORES)))
    global _last_results
    _last_results = res
    out = np.concatenate([res.results[i]["out"] for i in range(NCORES)], axis=0)
    return out


_last_results = None


if __name__ == "__main__":
    rng = np.random.default_rng(0)
    inputs = {
        "hidden": rng.standard_normal((1, B, D), dtype=np.float32),
        "encoder_outputs": rng.standard_normal((S, B, E2), dtype=np.float32),
        "W": (rng.standard_normal((D, E2 + D)) * 0.02).astype(np.float32),
        "b": (rng.standard_normal((D,)) * 0.02).astype(np.float32),
        "v": rng.random((D,), dtype=np.float32),
    }
    out = kernel(**inputs)
    print("out", out.shape, out.dtype, out.sum())


# revision 11
# speedup vs baseline: 1.0399x; 1.0034x over previous
"""Cross-attention (Bahdanau-style) scores kernel for 8 Trainium2 NeuronCores.

Reference computation (per batch b, source position s):
    energy[b,s,:] = tanh(Wh @ h[b] + We @ eo[s,b] + bias)
    scores[b,s]   = v . energy[b,s,:]
    out[b,:]      = softmax(scores[b,:])   over s

Sharding: data-parallel over batch (64 batches -> 8 per core). Weights are
replicated. No collectives needed (softmax is per-batch, fully local).

Per-core pipeline (S=4096, Bc=8, E2=512, D=256), v4:
  - host pre-transposes eo to [bb, g, h, p, c, j] bf16 so every DMA is a
    fully contiguous 512 KiB block (4 KiB per partition -> near-peak HBM BW)
  - DMA issue spread across Vector/Scalar/Sync queues so the first tiles
    land ~6.5us instead of ~15us (Sync alone has a 7us runtime preamble)
  - dummy matmuls on a memset tile warm the PE clock (HAM) from ~5us
  - PE matmul (bf16, N=512): PSUM [128, 2, 512], pep bufs=3
  - ACT: energy = tanh(eprojT + baseT[k]) fused, one instr per 1024 cols
  - PE dot (lag-3 slots, pair-batched, flushed per group):
    scores[8, s512] += vm[k,bb].T @ energy[k, s]
  - eager DVE copies PSUM->SBUF + eager ACT exp per 512-range
  - tail: last exp, 1/sum, scale split across ACT+DVE, two output DMAs.
"""

import numpy as np
import ml_dtypes

import concourse.bass as bass
import concourse.bacc as bacc
import concourse.tile as tile
from concourse import mybir
from concourse.bass_utils import run_bass_kernel_spmd

dt = mybir.dt

S = 4096          # src_len
B = 64            # global batch
E2 = 512          # 2*enc_hid
D = 256           # dec_hid
NCORES = 8
BC = B // NCORES  # batches per core = 8
P = 128
SG = 1024         # per-(bb,g) s-extent
NG = S // SG      # 4 s-groups
HW = 512          # matmul free-dim width (PSUM bank limit)
NH = SG // HW     # 2 halves per s-group
NR = S // HW      # 8 score ranges
NEC = E2 // P     # 4 e-chunks
NKC = D // P      # 2 k-chunks
LAG = 3           # dot emission lag in (bb,kc) slots
NWARM = 16        # PE warm-up dummy matmuls (bridge until data lands)

F32 = dt.float32
BF16 = dt.bfloat16


def build_program():
    nc = bacc.Bacc(None, target_bir_lowering=False, debug=False, num_devices=8)

    # eoT[bb, g, h, p, c, j] = eo[g*1024+h*512+j, bb, c*128+p]  (bf16)
    eoT_d = nc.declare_dram_parameter(
        "eoT", [BC, NG, NH, P, NEC, HW], BF16, isOutput=False)
    # WeT_r[p, ec, k] = We.T[ec*128+p, k] ; We = W[:, D:]
    weT_d = nc.declare_dram_parameter("weT", [P, NEC, D], BF16, isOutput=False)
    # WhT_r[p, dc, kc, j] = W[kc*128+j, dc*128+p]  (Wh part, pre-chunked)
    whT_d = nc.declare_dram_parameter("whT", [P, NKC, NKC, P], F32, isOutput=False)
    # hT[p, dc, bb] = h[bb, dc*128+p]
    hT_d = nc.declare_dram_parameter("hT", [P, NKC, BC], F32, isOutput=False)
    # bT[p, kc] = bias[kc*128+p]
    bT_d = nc.declare_dram_parameter("bT", [P, NKC], F32, isOutput=False)
    # vm[p, kc, bb, m] = v[kc*128+p] if m == bb else 0  (dot -> partition bb)
    vm_d = nc.declare_dram_parameter("vm", [P, NKC, BC, BC], BF16, isOutput=False)
    out_d = nc.declare_dram_parameter("out", [BC, S], F32, isOutput=True)

    with tile.TileContext(nc) as tc:
        with (
            tc.tile_pool(name="consts", bufs=1) as consts,
            tc.tile_pool(name="eot", bufs=6) as eot_pool,
            tc.tile_pool(name="en", bufs=12) as en_pool,
            tc.tile_pool(name="pep", bufs=3, space="PSUM") as pep_pool,
            tc.tile_pool(name="psc", bufs=2, space="PSUM") as psc_pool,
        ):
            # ---- PE warm-up: memset a tile, run dummy matmuls on it ----
            warm = consts.tile([P, HW], BF16)
            nc.vector.memset(warm, 0)
            ps_warm = psc_pool.tile([BC, HW], F32, tag="psc", name="ps_warm")
            for _ in range(NWARM):
                nc.tensor.matmul(ps_warm, warm[:, :BC], warm,
                                 start=True, stop=True)

            # ---- DMAs spread across queues for early arrival ----
            whT = consts.tile([P, NKC, NKC, P], F32)
            nc.scalar.dma_start(out=whT, in_=whT_d[:])
            hT = consts.tile([P, NKC, BC], F32)
            nc.scalar.dma_start(out=hT, in_=hT_d[:])
            eoT_first = eot_pool.tile([P, NH, NEC, HW], BF16, tag="eot",
                                      name="eoT_first")
            nc.gpsimd.dma_start(out=eoT_first[:, 0], in_=eoT_d[0, 0, 0])
            weT = consts.tile([P, NEC, D], BF16)
            nc.gpsimd.dma_start(out=weT, in_=weT_d[:])
            vm = consts.tile([P, NKC, BC, BC], BF16)
            nc.sync.dma_start(out=vm, in_=vm_d[:])
            bT = consts.tile([P, NKC], F32)
            nc.sync.dma_start(out=bT, in_=bT_d[:])
            nc.sync.dma_start(out=eoT_first[:, 1], in_=eoT_d[0, 0, 1])

            baseT = consts.tile([P, NKC, BC], F32)    # [k128, kc, bb]
            esums = consts.tile([BC, NR], F32)        # per-range exp sums
            out_sb = consts.tile([BC, S], F32)

            # --- init: baseT[k, bb] = sum_d Wh[k, d] h[bb, d] + bias[k] ---
            ps_base = pep_pool.tile([P, NKC, BC], F32, tag="pep",
                                    name="ps_base")
            for kc in range(NKC):
                for dc in range(NKC):
                    nc.tensor.matmul(
                        ps_base[:, kc, :],
                        whT[:, dc, kc, :],
                        hT[:, dc, :],
                        start=(dc == 0),
                        stop=(dc == NKC - 1),
                    )
            for kc in range(NKC):
                nc.vector.tensor_scalar_add(
                    baseT[:, kc, :], ps_base[:, kc, :], bT[:, kc : kc + 1]
                )

            pending = []  # (ps_sc, en_half_ap, kc, bb, first, last, ridx)

            def emit_dot(item):
                ps_sc, en_ap, kc, bb, first, last, ridx = item
                nc.tensor.matmul(
                    ps_sc, vm[:, kc, bb, :], en_ap, start=first, stop=last,
                )
                if last:
                    # eager: exp straight out of PSUM (frees the bank, keeps
                    # the tail chain short)
                    nc.scalar.activation(
                        out=out_sb[:, ridx * HW : (ridx + 1) * HW],
                        in_=ps_sc,
                        func=mybir.ActivationFunctionType.Exp,
                        accum_out=esums[:, ridx : ridx + 1],
                    )

            for g in range(NG):
                ps_scs = [psc_pool.tile([BC, HW], F32, tag="psc",
                                        name=f"psc_{g}_{h}")
                          for h in range(NH)]
                for bb in range(BC):
                    if g == 0 and bb == 0:
                        eoT_t = eoT_first
                    else:
                        eoT_t = eot_pool.tile([P, NH, NEC, HW], BF16,
                                              tag="eot", name=f"eoT_{g}_{bb}")
                        if g == 0 and bb <= 4:
                            ring = [nc.scalar, nc.gpsimd, nc.sync]
                            for h in range(NH):
                                eng = ring[(NH * bb + h) % 3]
                                eng.dma_start(out=eoT_t[:, h],
                                              in_=eoT_d[bb, g, h])
                        else:
                            eng = nc.sync if (bb % 2 == 0) else nc.gpsimd
                            for h in range(NH):
                                eng.dma_start(out=eoT_t[:, h],
                                              in_=eoT_d[bb, g, h])

                    # ---- projection + fused tanh; dots emitted lagged ----
                    for kc in range(NKC):
                        ps_ep = pep_pool.tile([P, NH, HW], F32, tag="pep",
                                              name=f"pep_{g}_{bb}_{kc}")
                        for c in range(NEC):
                            for h in range(NH):
                                nc.tensor.matmul(
                                    ps_ep[:, h, :],
                                    weT[:, c, kc * P : (kc + 1) * P],
                                    eoT_t[:, h, c, :],
                                    start=(c == 0),
                                    stop=(c == NEC - 1),
                                )
                        en = en_pool.tile([P, SG], BF16, tag="en",
                                          name=f"en_{g}_{bb}_{kc}")
                        nc.scalar.activation(
                            out=en, in_=ps_ep,
                            func=mybir.ActivationFunctionType.Tanh,
                            bias=baseT[:, kc, bb : bb + 1],
                        )
                        first = bb == 0 and kc == 0
                        last = bb == BC - 1 and kc == NKC - 1
                        for h in range(NH):
                            pending.append(
                                (ps_scs[h], en[:, h * HW : (h + 1) * HW],
                                 kc, bb, first, last, g * NH + h)
                            )
                        # batch-8: drain eight slots' dots at once (fewer
                        # stationary-operand switches between weT and vm)
                        if len(pending) >= (LAG + 8) * NH:
                            for _ in range(8 * NH):
                                emit_dot(pending.pop(0))
                # group boundary: flush this group's dots (frees psc banks
                # well before the next-but-one group needs them)
                while pending:
                    emit_dot(pending.pop(0))

            # ---- softmax tail ----
            with tc.tile_pool(name="sm", bufs=1) as sm:
                esum = sm.tile([BC, 1], F32)
                nc.vector.tensor_reduce(
                    out=esum, in_=esums, axis=mybir.AxisListType.X,
                    op=mybir.AluOpType.add,
                )
                rsum = sm.tile([BC, 1], F32)
                nc.vector.reciprocal(rsum, esum)
                # scale quarters on two engines in parallel, DMA each ASAP
                Q = S // 4
                for qi, eng_scale in ((0, "act"), (2, "dve"), (1, "act"),
                                      (3, "dve")):
                    sl = slice(qi * Q, (qi + 1) * Q)
                    if eng_scale == "act":
                        nc.scalar.activation(
                            out=out_sb[:, sl], in_=out_sb[:, sl],
                            func=mybir.ActivationFunctionType.Copy,
                            scale=rsum,
                        )
                    else:
                        nc.vector.tensor_scalar_mul(
                            out_sb[:, sl], out_sb[:, sl], rsum
                        )
                    dma_eng = nc.sync if qi % 2 == 0 else nc.gpsimd
                    dma_eng.dma_start(out=out_d[:, sl], in_=out_sb[:, sl])

    return nc


_nc = None


def _get_nc():
    global _nc
    if _nc is None:
        _nc = build_program()
        _nc.compile()
    return _nc


def kernel(hidden, encoder_outputs, W, b, v):
    hidden = np.asarray(hidden, dtype=np.float32)
    encoder_outputs = np.ascontiguousarray(encoder_outputs, dtype=np.float32)
    W = np.asarray(W, dtype=np.float32)
    b = np.asarray(b, dtype=np.float32)
    v = np.asarray(v, dtype=np.float32)

    # host-side prep of the small replicated weights
    We = W[:, D:]                                     # [256, 512]
    weT = np.ascontiguousarray(
        We.T.reshape(NEC, P, D).transpose(1, 0, 2)    # [p, ec, k]
    ).astype(ml_dtypes.bfloat16)
    # whT[p, dc, kc, j] = W[kc*128+j, dc*128+p]
    Wh = W[:, :D]                                     # [k, d]
    whT = np.ascontiguousarray(
        Wh.reshape(NKC, P, NKC, P).transpose(3, 2, 0, 1)  # [p(d), dc, kc, j(k)]
    )
    bT = np.ascontiguousarray(b.reshape(NKC, P).T)    # [p, kc]
    vT = np.ascontiguousarray(v.reshape(NKC, P).T)
    vm = np.zeros((P, NKC, BC, BC), dtype=np.float32)
    for bb in range(BC):
        vm[:, :, bb, bb] = vT
    vm = vm.astype(ml_dtypes.bfloat16)
    h = hidden[0]                                     # [64, 256]

    nc = _get_nc()
    eo_bf16 = encoder_outputs.astype(ml_dtypes.bfloat16)
    # [S, B, E2] -> [B, E2, S]
    eoT_full = np.ascontiguousarray(eo_bf16.transpose(1, 2, 0))
    in_maps = []
    for i in range(NCORES):
        bsl = slice(i * BC, (i + 1) * BC)
        hT_i = np.ascontiguousarray(h[bsl].T.reshape(NKC, P, BC).transpose(1, 0, 2))
        # [BC, NEC, P, NG, NH, HW] -> [BC, NG, NH, P, NEC, HW]
        eoT_i = np.ascontiguousarray(
            eoT_full[bsl].reshape(BC, NEC, P, NG, NH, HW)
            .transpose(0, 3, 4, 2, 1, 5)
        )
        in_maps.append(
            {"eoT": eoT_i, "weT": weT, "whT": whT, "hT": hT_i, "bT": bT,
             "vm": vm}
        )

    try:
        res = run_bass_kernel_spmd(nc, in_maps, list(range(NCORES)))
    except Exception:
        # transient NRT/device hiccups happen; one retry
        res = run_bass_kernel_spmd(nc, in_maps, list(range(NCORES)))
    global _last_results
    _last_results = res
    out = np.concatenate([res.results[i]["out"] for i in range(NCORES)], axis=0)
    return out


_last_results = None


if __name__ == "__main__":
    rng = np.random.default_rng(0)
    inputs = {
        "hidden": rng.standard_normal((1, B, D), dtype=np.float32),
        "encoder_outputs": rng.standard_normal((S, B, E2), dtype=np.float32),
        "W": (rng.standard_normal((D, E2 + D)) * 0.02).astype(np.float32),
        "b": (rng.standard_normal((D,)) * 0.02).astype(np.float32),
        "v": rng.random((D,), dtype=np.float32),
    }
    out = kernel(**inputs)
    print("out", out.shape, out.dtype, out.sum())
